# revision 76
# baseline (speedup 1.0000x reference)
"""GAT link-prediction kernel for Trainium2, 8-core SPMD.  v2

Strategy (graph/data parallel per the dst-owner sharding hint):
- Nodes are relabeled by in-degree and dealt round-robin to 8 cores; the
  global node table uses a CHUNK-MAJOR layout (4 tile-chunks x 8 cores) so
  each AllGather chunk lands contiguously and the table splits into two
  int16-addressable halves at a chunk boundary.
- Per GAT layer: node phase computes bf16 [h@W | h@a_sW | h@a_dW] rows
  (512B each); chunked AllGather replicates the table; the edge phase
  gathers 512B rows per edge in bucketed-ELL form (dst-partition layout),
  does a masked segment softmax on the free axis, and accumulates
  alpha-weighted rows with a DVE/PE split (diag-scaled matmuls on the PE
  path).  Self-loops never touch the gather path: their contribution is
  computed from the local node-phase tile.
- All per-edge metadata (indices, masks, GCN norms) is SBUF-resident and
  shared by all layers; h stays on-chip between layers.
- GCN layer gathers f32 z rows (256B); decode groups label edges by
  (src-half, dst-half) and dots gathered z rows.
"""
import numpy as np
from concourse import bass, bacc, mybir, tile, bass_utils

NCORES = 8
N = 50000
IN = 128
HID = 128
OUT = 64
NL = 200000
NEG = 0.2

SP = 6272                 # padded nodes per core (49 * 128)
NT = SP // 128            # 49 tiles per core
G = NCORES * SP           # 50176 global table rows
# AllGather chunks by tile range; table layout is chunk-major:
# rows [CBASE[q] + c*CROWS[q] + r) for core c, within-chunk row r
CH = [(0, 7), (7, 13), (13, 19), (19, 25),
      (25, 31), (31, 37), (37, 43), (43, 49)]
CROWS = [(b - a) * 128 for a, b in CH]
CBASE = [0]
for r in CROWS:
    CBASE.append(CBASE[-1] + NCORES * r)
B0 = CBASE[4]             # 25600: gather half 0 = chunks 0..3
B1 = G - B0               # 24576: gather half 1 = chunks 2,3
WG = 256                  # GAT table row bf16: h(128)|hs|hd|pad  (512B)
WZ = 64                   # z table row f32 (256B)
TWG = 130                 # packed SBUF row: h|hs|hd
KCAP = 12                 # max ELL slots per gather call
PB = 1024                 # decode gather batch (indices)
PBC = PB // 128           # 16 label-tile chunks per batch
PE_FRAC_GAT = 46          # % of MAC slots on the PE (diag-matmul) path
PO_FRAC_GAT = 0          # % of MAC slots on the gpsimd path
PE_FRAC_GCN = 35

f32 = mybir.dt.float32
bf16 = mybir.dt.bfloat16
i16 = mybir.dt.int16


def _wrap16(flat):
    """dma_gather index layout: value at [j%16, j//16], replicated to all
    8 gpsimd core groups -> [128, n//16] int16."""
    n = len(flat)
    cols = n // 16
    blk = np.ascontiguousarray(flat.astype(np.int16).reshape(cols, 16).T)
    return np.tile(blk, (8, 1))


def _prep(x, ei, eli, W1, a1s, a1d, b1, W2, a2s, a2d, b2,
          W3, a3s, a3d, b3, W4, b4):
    src = np.asarray(ei[0], np.int64)
    dst = np.asarray(ei[1], np.int64)

    deg = np.bincount(dst, minlength=N) + 1          # with self-loop
    order = np.argsort(-deg, kind="stable")          # global degree order
    core = np.empty(N, np.int64)
    core[order] = np.arange(N) % NCORES

    # fixed half membership: top 3200 in-degree nodes of each core -> tiles
    # 0..24 (table rows < B0); remainder -> tiles 25..48
    H0T = CH[3][1] * 128                             # 3200 local rows in half0
    lrank_deg = np.empty(N, np.int64)
    for c in range(NCORES):
        nodes = order[core[order] == c]              # deg desc within core
        lrank_deg[nodes] = np.arange(len(nodes))
    hf = (lrank_deg >= H0T).astype(np.int64)         # node's half as src

    # per-node in-edge counts by src half (self-loops excluded from ELL)
    c0n = np.bincount(dst[hf[src] == 0], minlength=N)
    c1n = np.bincount(dst[hf[src] == 1], minlength=N)

    # within-half ordering: c0 desc, snake 768-blocks by c1 (tile tightness)
    lrank = np.empty(N, np.int64)
    for c in range(NCORES):
        for half in (0, 1):
            nodes = np.where((core == c) & (hf == half))[0]
            o = nodes[np.lexsort((-c1n[nodes], -c0n[nodes]))]
            parts = []
            for i in range(0, len(o), 768):
                blk = o[i:i + 768]
                parts.append(blk[np.argsort(-c1n[blk], kind="stable")])
            o = np.concatenate(parts)
            lrank[o] = half * H0T + np.arange(len(o))

    # global (chunk-major) table row per node
    tile_of = lrank // 128
    chunk_of = np.searchsorted([c[1] for c in CH], tile_of, side="right")
    crows = np.asarray(CROWS, np.int64)
    cbase = np.asarray(CBASE[:len(CH)], np.int64)
    cstart = np.asarray([c[0] * 128 for c in CH], np.int64)
    grow = cbase[chunk_of] + core * crows[chunk_of] + (lrank - cstart[chunk_of])

    deg_g = np.zeros(G, np.int64)
    deg_g[grow] = deg
    dinv = np.zeros(G, np.float64)
    nz = deg_g > 0
    dinv[nz] = 1.0 / np.sqrt(deg_g[nz])

    # ---- ELL grids (per core, self-loops excluded) ----
    S = grow[src]
    D = grow[dst]
    cD = core[dst]
    half_s = (S >= B0).astype(np.int64)
    loc16 = S - half_s * B0
    # sort edges by (dst core, dst lrank, src half)
    key = (cD * SP + lrank[dst]) * 2 + half_s
    sidx = np.argsort(key, kind="stable")
    ks = key[sidx]
    loc_s = loc16[sidx]
    Ss = S[sidx]
    cnt = np.bincount(key, minlength=2 * NCORES * SP)
    startp = np.zeros(2 * NCORES * SP + 1, np.int64)
    np.cumsum(cnt, out=startp[1:])
    slot = np.arange(len(ks), dtype=np.int64) - startp[ks]

    c0 = cnt[0::2].reshape(NCORES, NT, 128)
    c1 = cnt[1::2].reshape(NCORES, NT, 128)
    K0 = np.maximum(c0.max(axis=(0, 2)), 1).astype(int)
    K1 = np.maximum(c1.max(axis=(0, 2)), 1).astype(int)
    K0m, K1m = int(K0.max()), int(K1.max())

    e0 = (ks % 2) == 0
    e1 = ~e0
    lin = ks // 2                                    # core*SP + lrank of dst
    grid0 = np.zeros((NCORES * SP, K0m), np.int16)
    vm0 = np.zeros((NCORES * SP, K0m), bool)
    grid0[lin[e0], slot[e0]] = loc_s[e0].astype(np.int16)
    vm0[lin[e0], slot[e0]] = True
    grid1 = np.zeros((NCORES * SP, K1m), np.int16)
    vm1 = np.zeros((NCORES * SP, K1m), bool)
    grid1[lin[e1], slot[e1]] = loc_s[e1].astype(np.int16)
    vm1[lin[e1], slot[e1]] = True
    nval = (dinv[Ss] * dinv[grow[dst[sidx]]]).astype(np.float32)
    nw0 = np.zeros((NCORES * SP, K0m), np.float32)
    nw0[lin[e0], slot[e0]] = nval[e0]
    nw1 = np.zeros((NCORES * SP, K1m), np.float32)
    nw1[lin[e1], slot[e1]] = nval[e1]

    import ml_dtypes

    def tobf(a):
        return np.asarray(a, np.float32).astype(ml_dtypes.bfloat16)

    # ---- permuted/padded node features ----
    x = np.asarray(x, np.float32)
    xg = np.zeros((NCORES, SP, IN), np.float32)
    xg[core, lrank] = x

    def pack(W, as_, ad_):
        W = np.asarray(W, np.float32)
        out = np.zeros((IN, WG), np.float32)
        out[:, :HID] = W
        out[:, HID] = W @ np.asarray(as_, np.float32)
        out[:, HID + 1] = W @ np.asarray(ad_, np.float32)
        return tobf(out)
    wx = [pack(W1, a1s, a1d), pack(W2, a2s, a2d), pack(W3, a3s, a3d)]
    w4 = tobf(np.asarray(W4, np.float32))
    bias = [np.asarray(b, np.float32).reshape(1, -1) for b in (b1, b2, b3, b4)]

    # ---- decode: shard label edges by position, group by (halfA, halfB) ----
    A = grow[np.asarray(eli[0], np.int64)]
    B = grow[np.asarray(eli[1], np.int64)]
    npc = NL // NCORES
    gidx = [(A[c * npc:(c + 1) * npc] >= B0) * 2 +
            (B[c * npc:(c + 1) * npc] >= B0) for c in range(NCORES)]
    gcounts = np.array([np.bincount(g, minlength=4) for g in gidx])
    NBg = [int(-(-gcounts[:, g].max() // PB)) for g in range(4)]
    TOTB = sum(NBg)

    in_maps = []
    unshard = []
    KT = [int(a + b) for a, b in zip(K0, K1)]
    SK = int(sum(KT))
    SK0 = int(sum(K0))
    SK1 = int(sum(K1))
    for c in range(NCORES):
        rows = slice(c * SP, (c + 1) * SP)
        ix0p, ix1p, mkp, nwp = [], [], [], []
        for t in range(NT):
            r = slice(c * SP + t * 128, c * SP + (t + 1) * 128)
            k0, k1 = K0[t], K1[t]
            f0 = np.ascontiguousarray(grid0[r, :k0].T).reshape(-1)
            f1 = np.ascontiguousarray(grid1[r, :k1].T).reshape(-1)
            ix0p.append(_wrap16(f0))                 # [128, 8*k0]
            ix1p.append(_wrap16(f1))
            m = np.full((128, k0 + k1), np.float32(-1e30), np.float32)
            m[:, :k0][vm0[r, :k0]] = 0.0
            m[:, k0:][vm1[r, :k1]] = 0.0
            mkp.append(m)
            w = np.concatenate([nw0[r, :k0], nw1[r, :k1]], axis=1)
            nwp.append(np.ascontiguousarray(w))

        # per-node self norm dinv^2, laid out [128, NT]
        gr_c = np.zeros(SP, np.int64)
        nodes_c = np.where(core == c)[0]
        gr_c[lrank[nodes_c]] = grow[nodes_c]
        nws = (dinv[gr_c] ** 2).astype(np.float32).reshape(NT, 128).T
        nws = np.ascontiguousarray(nws)

        Ac, Bc = A[c * npc:(c + 1) * npc], B[c * npc:(c + 1) * npc]
        gc = gidx[c]
        ordc = np.argsort(gc, kind="stable")
        diap, dibp = [], []
        for g in range(4):
            sel = ordc[gc[ordc] == g]
            na = NBg[g] * PB
            av = np.zeros(na, np.int64)
            bv = np.zeros(na, np.int64)
            av[:len(sel)] = Ac[sel] - (g >> 1) * B0
            bv[:len(sel)] = Bc[sel] - (g & 1) * B0
            for nb in range(NBg[g]):
                diap.append(_wrap16(av[nb * PB:(nb + 1) * PB]).reshape(-1))
                dibp.append(_wrap16(bv[nb * PB:(nb + 1) * PB]).reshape(-1))

        im = {
            "xs": tobf(np.ascontiguousarray(
                xg[c].reshape(NT, 128, IN).transpose(1, 0, 2).reshape(
                    128, NT * IN))),
            "ix0": np.concatenate(ix0p, axis=1),
            "ix1": np.concatenate(ix1p, axis=1),
            "msk": tobf(np.concatenate(mkp, axis=1)),
            "nwt": np.concatenate(nwp, axis=1),
            "nws": nws,
            "dia": np.concatenate(diap), "dib": np.concatenate(dibp),
            "wx1": wx[0], "wx2": wx[1], "wx3": wx[2], "w4p": w4,
            "bi1": bias[0], "bi2": bias[1], "bi3": bias[2], "bi4": bias[3],
        }
        in_maps.append(im)
        unshard.append(ordc)

    prof = {
        "K0": [int(k) for k in K0], "K1": [int(k) for k in K1],
        "NBg": NBg, "TOTB": TOTB,
        "SK": SK, "SK0": SK0, "SK1": SK1,
    }
    meta = {"gcounts": gcounts, "npc": npc}
    return prof, in_maps, unshard, meta


def _build(prof, sim_mode=False, ablate=()):
    K0, K1 = prof["K0"], prof["K1"]
    NBg, TOTB = prof["NBg"], prof["TOTB"]
    SK, SK0, SK1 = prof["SK"], prof["SK0"], prof["SK1"]
    KT = [a + b for a, b in zip(K0, K1)]
    KTM = max(KT)
    AluOp = mybir.AluOpType
    Act = mybir.ActivationFunctionType

    nc = bacc.Bacc("TRN2", target_bir_lowering=False, debug=False,
                   num_devices=NCORES, dynamic_dma_scratch_size=32768)

    xs = nc.dram_tensor("xs", [128, NT * IN], bf16, kind="ExternalInput")
    wxh = [nc.dram_tensor(f"wx{l}", [IN, WG], bf16, kind="ExternalInput")
           for l in (1, 2, 3)]
    w4h = nc.dram_tensor("w4p", [HID, WZ], bf16, kind="ExternalInput")
    bih = [nc.dram_tensor(f"bi{l}", [1, HID if l < 4 else WZ], f32,
                          kind="ExternalInput") for l in (1, 2, 3, 4)]
    ix0h = nc.dram_tensor("ix0", [128, 8 * SK0], i16, kind="ExternalInput")
    ix1h = nc.dram_tensor("ix1", [128, 8 * SK1], i16, kind="ExternalInput")
    mskh = nc.dram_tensor("msk", [128, SK], bf16, kind="ExternalInput")
    nwth = nc.dram_tensor("nwt", [128, SK], f32, kind="ExternalInput")
    nwsh = nc.dram_tensor("nws", [128, NT], f32, kind="ExternalInput")
    diah = nc.dram_tensor("dia", [TOTB * PB * 8], i16, kind="ExternalInput")
    dibh = nc.dram_tensor("dib", [TOTB * PB * 8], i16, kind="ExternalInput")
    outh = nc.dram_tensor("logits", [TOTB, 128, PBC], f32,
                          kind="ExternalOutput")

    tsh = [nc.dram_tensor(f"tsh{l}", [SP, WG], bf16, kind="Internal")
           for l in (1, 2, 3)]
    tab = [nc.dram_tensor(f"tab{l}", [G, WG], bf16, kind="Internal",
                          addr_space="Shared") for l in (1, 2, 3)]
    tshz = nc.dram_tensor("tshz", [SP, WZ], f32, kind="Internal")
    tabz = nc.dram_tensor("tabz", [G, WZ], f32, kind="Internal",
                          addr_space="Shared")
    zout = nc.dram_tensor("zout", [SP, WZ], f32, kind="Internal")
    ztab = nc.dram_tensor("ztab", [G, WZ], f32, kind="Internal",
                          addr_space="Shared")

    # per-tile column offsets into resident metadata
    off0 = np.concatenate([[0], np.cumsum(K0)]).astype(int)
    off1 = np.concatenate([[0], np.cumsum(K1)]).astype(int)
    offm = np.concatenate([[0], np.cumsum(KT)]).astype(int)
    tile_chunk = {}
    for q, (a, b) in enumerate(CH):
        for t in range(a, b):
            tile_chunk[t] = q

    def flat_ap(handle, off, p, q):
        return bass.AP(bass.DRamTensorHandle(handle.name, list(handle.shape),
                                             handle.dtype),
                       int(off), [[q, p], [1, q]])

    from concourse.masks import make_identity

    rg = [list(range(NCORES))]

    with tile.TileContext(nc) as tc:
        with tc.tile_pool(name="const", bufs=1) as cp, \
             tc.tile_pool(name="psum", bufs=2, space="PSUM") as pp, \
             tc.tile_pool(name="sb", bufs=4) as sb, \
             tc.tile_pool(name="gath", bufs=16) as gp:

            identb = cp.tile([128, 128], bf16, tag="identb")
            make_identity(nc, identb[:])
            identf = cp.tile([128, 128], f32, tag="identf")
            make_identity(nc, identf[:])
            ones1 = cp.tile([1, 128], f32, tag="ones1")
            nc.vector.memset(ones1[:], 1.0)
            nb20 = cp.tile([128, 1], f32, tag="nb20")
            nc.vector.memset(nb20[:], -20.0)
            zro = cp.tile([128, 128], f32, tag="zro")
            nc.vector.memset(zro[:], 0.0)

            wt = []
            for l in (1, 2, 3):
                w = cp.tile([128, WG], bf16, tag=f"wx{l}")
                nc.sync.dma_start(out=w[:], in_=wxh[l - 1].ap())
                wt.append(w)
            w4t = cp.tile([128, WZ], bf16, tag="w4t")
            nc.sync.dma_start(out=w4t[:], in_=w4h.ap())

            bb = []
            for l in (1, 2, 3, 4):
                wdt = HID if l < 4 else WZ
                bs = sb.tile([1, wdt], f32, tag="bld")
                nc.sync.dma_start(out=bs[:], in_=bih[l - 1].ap())
                bps = pp.tile([128, wdt], f32, tag="mm")
                nc.tensor.matmul(bps[:], lhsT=ones1[:], rhs=bs[:],
                                 start=True, stop=True)
                bt = cp.tile([128, wdt], f32, tag=f"bb{l}")
                nc.vector.tensor_copy(bt[:], bps[:])
                bb.append(bt)

            # x first so the layer-1 node phase starts before the
            # edge-phase metadata loads
            hA = cp.tile([128, NT * HID], bf16, tag="hA")
            nc.sync.dma_start(out=hA[:], in_=xs.ap())
            ixr0 = cp.tile([128, 8 * SK0], i16, tag="ixr0")
            nc.sync.dma_start(out=ixr0[:], in_=ix0h.ap())
            ixr1 = cp.tile([128, 8 * SK1], i16, tag="ixr1")
            nc.sync.dma_start(out=ixr1[:], in_=ix1h.ap())
            mskr = cp.tile([128, SK], bf16, tag="mskr")
            nc.sync.dma_start(out=mskr[:], in_=mskh.ap())
            nwtr = cp.tile([128, SK], f32, tag="nwtr")
            nwsr = cp.tile([128, NT], f32, tag="nwsr")
            nc.sync.dma_start(out=nwsr[:], in_=nwsh.ap())
            tshS = cp.tile([128, NT * TWG], bf16, tag="tshS")
            diar = cp.tile([128, TOTB * (PB // 16)], i16, tag="diar")
            zS = cp.tile([128, NT * WZ], f32, tag="zS")

            def node_tile(l, t, hsrc):
                """hsrc: [128,128] bf16 AP of layer input tile t."""
                tp = pp.tile([128, 128], bf16, tag="tp")
                nc.tensor.transpose(tp[:], hsrc, identb[:])
                hTs = sb.tile([128, 128], bf16, tag="hTs")
                nc.scalar.activation(hTs[:], tp[:], Act.Copy)
                wdt = WG if l < 4 else WZ
                mm = pp.tile([128, wdt], f32, tag="mm")
                nc.tensor.matmul(mm[:], lhsT=hTs[:],
                                 rhs=(wt[l - 1] if l < 4 else w4t)[:],
                                 start=True, stop=True)
                if l < 4:
                    dstS = tshS[:, t * TWG:(t + 1) * TWG]
                    nc.scalar.activation(dstS, mm[:, :TWG], Act.Copy)
                    nc.sync.dma_start(
                        out=tsh[l - 1].ap()[t * 128:(t + 1) * 128, :TWG],
                        in_=dstS)
                else:
                    dstS = zS[:, t * WZ:(t + 1) * WZ]
                    nc.scalar.activation(dstS, mm[:], Act.Copy)
                    nc.sync.dma_start(
                        out=tshz.ap()[t * 128:(t + 1) * 128, :], in_=dstS)

            def emit_ag(srch, dsth, q, wdt):
                r0 = CH[q][0] * 128
                rows = CROWS[q]
                if sim_mode:
                    for cc in range(NCORES):
                        base = CBASE[q] + cc * rows
                        nc.sync.dma_start(
                            out=dsth.ap()[base:base + rows, :],
                            in_=srch.ap()[r0:r0 + rows, :])
                else:
                    nc.gpsimd.collective_compute(
                        "AllGather", AluOp.bypass, replica_groups=rg,
                        ins=[srch.ap()[r0:r0 + rows, :]],
                        outs=[dsth.ap()[CBASE[q]:CBASE[q] + NCORES * rows, :]])

            def gat_gathers(l, t):
                chunks = []
                for ixr, offs, Kt, base, hlen, ab0 in (
                        (ixr0, off0, K0[t], 0, B0, 0),
                        (ixr1, off1, K1[t], B0, B1, K0[t])):
                    cs = 0
                    while cs < Kt:
                        cn = min(KCAP, Kt - cs)
                        gt = gp.tile([128, KCAP, WG], bf16, tag="g0")
                        nc.gpsimd.dma_gather(
                            out_ap=gt[:, :cn, :],
                            in_ap=tab[l - 1].ap()[base:base + hlen],
                            idxs_ap=ixr[:, 8 * (offs[t] + cs):
                                        8 * (offs[t] + cs + cn)],
                            num_idxs=128 * cn, num_idxs_reg=128 * cn,
                            elem_size=WG, single_packet=False)
                        chunks.append((gt, cn, ab0 + cs))
                        cs += cn
                return chunks

            def edge_tile_gat(l, t, hout):
                kt = KT[t]
                chunks = gat_gathers(l, t)
                hdcol = tshS[:, t * TWG + 129:t * TWG + 130]
                sc = sb.tile([128, KTM + 1], f32, tag="sc")
                # raw scores: (hs + hd + mask), then leaky
                for gt, cn, ab in chunks:
                    sch = sc[:, ab:ab + cn]
                    nc.vector.scalar_tensor_tensor(
                        out=sch, in0=gt[:, :cn, 128:129], scalar=hdcol,
                        in1=mskr[:, offm[t] + ab:offm[t] + ab + cn],
                        op0=AluOp.add, op1=AluOp.add)
                    nc.vector.scalar_tensor_tensor(
                        out=sch, in0=sch, scalar=NEG, in1=sch,
                        op0=AluOp.mult, op1=AluOp.max)
                # self-loop score -> column kt
                s0 = sc[:, kt:kt + 1]
                nc.vector.tensor_tensor(
                    out=s0, in0=tshS[:, t * TWG + 128:t * TWG + 129],
                    in1=hdcol, op=AluOp.add)
                nc.vector.scalar_tensor_tensor(
                    out=s0, in0=s0, scalar=NEG, in1=s0,
                    op0=AluOp.mult, op1=AluOp.max)
                # two shifted exps (first covers early chunks so their MACs
                # and gather buffers release sooner); accums -> softmax denom
                ssum = sb.tile([128, 1], f32, tag="sst")
                ss2 = sb.tile([128, 1], f32, tag="ss2")
                nch = len(chunks)
                cut = chunks[(nch + 1) // 2 - 1]
                cut = cut[2] + cut[1]          # abase + cn of middle chunk
                nc.scalar.activation(sc[:, :cut], sc[:, :cut], Act.Exp,
                                     bias=nb20[:, :1], accum_out=ssum[:])
                nc.scalar.activation(sc[:, cut:kt + 1], sc[:, cut:kt + 1],
                                     Act.Exp, bias=nb20[:, :1],
                                     accum_out=ss2[:])
                nc.vector.tensor_add(ssum[:], ssum[:], ss2[:])
                # acc init: self contribution (DVE, so the MAC chain is not
                # gated behind the Act diag-build queue)
                acc = sb.tile([128, HID], f32, tag="acc")
                nc.vector.scalar_tensor_tensor(
                    out=acc[:], in0=tshS[:, t * TWG:t * TWG + HID],
                    scalar=sc[:, kt:kt + 1], in1=zro[:],
                    op0=AluOp.mult, op1=AluOp.add)
                # MAC over slots: 3-way DVE / Act+PE / Pool split
                pe_n = po_n = 0
                if "agg" not in ablate:
                    slots = [(gt, k, ab + k) for gt, cn, ab in chunks
                             for k in range(cn)]
                    ns = len(slots)
                    pe_n = (ns * PE_FRAC_GAT) // 100
                    po_n = (ns * PO_FRAC_GAT) // 100
                    dve_slots = slots[:ns - pe_n - po_n]
                    po_slots = slots[ns - pe_n - po_n:ns - pe_n]
                    pe_slots = slots[ns - pe_n:]
                    for gt, k, ai in dve_slots:
                        nc.vector.scalar_tensor_tensor(
                            out=acc[:], in0=gt[:, k, :HID],
                            scalar=sc[:, ai:ai + 1], in1=acc[:],
                            op0=AluOp.mult, op1=AluOp.add)
                    if po_n:
                        accP = sb.tile([128, HID], f32, tag="accP")
                        nc.gpsimd.memset(accP[:], 0.0)
                        for gt, k, ai in po_slots:
                            nc.gpsimd.scalar_tensor_tensor(
                                out=accP[:], in0=gt[:, k, :HID],
                                scalar=sc[:, ai:ai + 1], in1=accP[:],
                                op0=AluOp.mult, op1=AluOp.add)
                    if pe_n:
                        pacc = pp.tile([128, HID], f32, tag="pacc")
                        for i, (gt, k, ai) in enumerate(pe_slots):
                            dg = sb.tile([128, 128], bf16, tag="dg")
                            nc.scalar.activation(dg[:], identb[:], Act.Copy,
                                                 scale=sc[:, ai:ai + 1])
                            nc.tensor.matmul(pacc[:], lhsT=dg[:],
                                             rhs=gt[:, k, :HID],
                                             start=(i == 0),
                                             stop=(i == pe_n - 1))
                # normalize + bias + relu -> hout tile (bf16)
                nc.vector.tensor_scalar_max(ssum[:], ssum[:], 1e-30)
                rr = sb.tile([128, 1], f32, tag="rr")
                nc.vector.reciprocal(rr[:], ssum[:])
                if po_n:
                    nc.vector.tensor_add(acc[:], acc[:], accP[:])
                if pe_n:
                    nc.vector.tensor_add(acc[:], acc[:], pacc[:])
                nc.vector.scalar_tensor_tensor(
                    out=acc[:], in0=acc[:], scalar=rr[:, :1],
                    in1=bb[l - 1][:], op0=AluOp.mult, op1=AluOp.add)
                nc.vector.tensor_scalar_max(hout, acc[:], 0.0)

            def edge_tile_gcn(t):
                chunks = []
                for ixr, offs, Kt, base, hlen, ab0 in (
                        (ixr0, off0, K0[t], 0, B0, 0),
                        (ixr1, off1, K1[t], B0, B1, K0[t])):
                    cs = 0
                    while cs < Kt:
                        cn = min(KCAP, Kt - cs)
                        gt = gp.tile([128, KCAP, WZ], f32, tag="g0")
                        nc.gpsimd.dma_gather(
                            out_ap=gt[:, :cn, :],
                            in_ap=tabz.ap()[base:base + hlen],
                            idxs_ap=ixr[:, 8 * (offs[t] + cs):
                                        8 * (offs[t] + cs + cn)],
                            num_idxs=128 * cn, num_idxs_reg=128 * cn,
                            elem_size=WZ, single_packet=False)
                        chunks.append((gt, cn, ab0 + cs))
                        cs += cn
                acc = sb.tile([128, WZ], f32, tag="accz")
                nc.vector.scalar_tensor_tensor(
                    out=acc[:], in0=zS[:, t * WZ:(t + 1) * WZ],
                    scalar=nwsr[:, t:t + 1], in1=zro[:, :WZ],
                    op0=AluOp.mult, op1=AluOp.add)
                if "agg" not in ablate:
                    slots = [(gt, k, ab + k) for gt, cn, ab in chunks
                             for k in range(cn)]
                    pe_n = (len(slots) * PE_FRAC_GCN) // 100
                    dve_slots = slots[:len(slots) - pe_n]
                    pe_slots = slots[len(slots) - pe_n:]
                    for gt, k, ai in dve_slots:
                        nc.vector.scalar_tensor_tensor(
                            out=acc[:], in0=gt[:, k, :WZ],
                            scalar=nwtr[:, offm[t] + ai:offm[t] + ai + 1],
                            in1=acc[:], op0=AluOp.mult, op1=AluOp.add)
                    if pe_n:
                        pacc = pp.tile([128, WZ], f32, tag="pacc")
                        for i, (gt, k, ai) in enumerate(pe_slots):
                            dg = sb.tile([128, 128], f32, tag="dgz")
                            nc.scalar.activation(
                                dg[:], identf[:], Act.Copy,
                                scale=nwtr[:, offm[t] + ai:offm[t] + ai + 1])
                            nc.tensor.matmul(pacc[:], lhsT=dg[:],
                                             rhs=gt[:, k, :WZ],
                                             start=(i == 0),
                                             stop=(i == pe_n - 1))
                        nc.vector.tensor_add(acc[:], acc[:], pacc[:])
                nc.vector.tensor_add(acc[:], acc[:], bb[3][:])
                nc.sync.dma_start(out=zout.ap()[t * 128:(t + 1) * 128, :],
                                  in_=acc[:])

            # ---- pipeline ----
            for t in range(NT):
                node_tile(1, t, hA[:, t * HID:(t + 1) * HID])
                q = tile_chunk[t]
                if t == CH[q][1] - 1:
                    emit_ag(tsh[0], tab[0], q, WG)

            for l in (1, 2, 3):
                hout_buf = hA
                for t in range(NT):
                    edge_tile_gat(l, t, hout_buf[:, t * HID:(t + 1) * HID])
                    node_tile(l + 1, t, hout_buf[:, t * HID:(t + 1) * HID])
                    q = tile_chunk[t]
                    if t == CH[q][1] - 1:
                        if l < 3:
                            emit_ag(tsh[l], tab[l], q, WG)
                        else:
                            emit_ag(tshz, tabz, q, WZ)
                if l == 2:
                    # decode A-side indices: load into the L2->L3 DMA hole
                    nc.sync.dma_start(
                        out=diar[:],
                        in_=bass.AP(bass.DRamTensorHandle(
                            diah.name, list(diah.shape), diah.dtype), 0,
                            [[PB // 16, 128], [PB * 8, TOTB],
                             [1, PB // 16]]))

            # GCN norms load here: fills the GAT3->GCN transition DMA hole
            nc.sync.dma_start(out=nwtr[:], in_=nwth.ap())
            for t in range(NT):
                edge_tile_gcn(t)
                q = tile_chunk[t]
                if t == CH[q][1] - 1:
                    emit_ag(zout, ztab, q, WZ)

            # ---- decode ----
            bi = 0
            for g in range(4 if "decode" not in ablate else 0):
                baseA = B0 * (g >> 1)
                hlenA = B1 if (g >> 1) else B0
                baseB = B0 * (g & 1)
                hlenB = B1 if (g & 1) else B0
                for _ in range(NBg[g]):
                    ia = diar[:, bi * (PB // 16):(bi + 1) * (PB // 16)]
                    ib = sb.tile([128, PB // 16], i16, tag="ib")
                    nc.sync.dma_start(
                        out=ib[:], in_=flat_ap(dibh, bi * PB * 8, 128,
                                               PB // 16))
                    ga = gp.tile([128, PBC, WZ], f32, tag="g0")
                    nc.gpsimd.dma_gather(
                        out_ap=ga[:], in_ap=ztab.ap()[baseA:baseA + hlenA],
                        idxs_ap=ia, num_idxs=PB, num_idxs_reg=PB,
                        elem_size=WZ, single_packet=False)
                    gb = gp.tile([128, PBC, WZ], f32, tag="g0")
                    nc.gpsimd.dma_gather(
                        out_ap=gb[:], in_ap=ztab.ap()[baseB:baseB + hlenB],
                        idxs_ap=ib[:], num_idxs=PB, num_idxs_reg=PB,
                        elem_size=WZ, single_packet=False)
                    pr = gp.tile([128, PBC, WZ], f32, tag="g0")
                    nc.vector.tensor_tensor(out=pr[:], in0=ga[:], in1=gb[:],
                                            op=AluOp.mult)
                    dt_ = sb.tile([128, PBC], f32, tag="dt")
                    nc.vector.tensor_reduce(dt_[:], pr[:],
                                            axis=mybir.AxisListType.X,
                                            op=AluOp.add)
                    nc.sync.dma_start(
                        out=bass.AP(bass.DRamTensorHandle(
                            outh.name, list(outh.shape), outh.dtype),
                            bi * 128 * PBC, [[PBC, 128], [1, PBC]]),
                        in_=dt_[:])
                    bi += 1

    nc.compile()
    return nc


def kernel(**inputs):
    prof, in_maps, unshard, meta = _prep(
        inputs["x"], inputs["edge_index"], inputs["edge_label_index"],
        inputs["W1"], inputs["a1s"], inputs["a1d"], inputs["b1"],
        inputs["W2"], inputs["a2s"], inputs["a2d"], inputs["b2"],
        inputs["W3"], inputs["a3s"], inputs["a3d"], inputs["b3"],
        inputs["W4"], inputs["b4"])
    nc = _build(prof)
    res = bass_utils.run_bass_kernel_spmd(
        nc, in_maps, core_ids=list(range(NCORES)))
    results = res.results

    npc = meta["npc"]
    NBg = prof["NBg"]
    gcounts = meta["gcounts"]
    out = np.empty(NL, np.float32)
    for c in range(NCORES):
        arr = results[c]["logits"]          # [TOTB, 128, PBC]
        flat = arr.transpose(0, 2, 1).reshape(-1)
        vals = []
        bi = 0
        for g in range(4):
            cnt = gcounts[c][g]
            vals.append(flat[bi * PB: bi * PB + cnt])
            bi += NBg[g]
        sorted_vals = np.concatenate(vals)
        block = np.empty(npc, np.float32)
        block[unshard[c]] = sorted_vals
        out[c * npc:(c + 1) * npc] = block
    return out
